# revision 1
# baseline (speedup 1.0000x reference)
"""Trainium2 Bass kernel for multi-head attention (dense transformer block).

Reference computation (per batch element):
    qkv = x @ w_qkv                      # [N, 3C]
    q, k, v = split heads (H=12, HD=64); q *= HD**-0.5
    out = softmax(q k^T) v               # full [N, N] scores
    out = merge_heads(out) @ w_proj + b_proj

Distribution: pure data parallel over the batch dim — B=8 batch elements,
8 NeuronCores, one element per core.  Weights are replicated.  No
collectives are needed; each core computes its full [2048, 768] output.

Per-core compute strategy (all matmuls bf16, fp32 PSUM accumulation):
  * x is cast f32->bf16 by a SWDGE DMA into a DRAM scratch, then DMA-xbar
    transposed into SBUF as xT [768, 2048] (feature-on-partition layout).
  * qkT = w_qk^T @ x^T -> [1536, 2048]: q/k for a head PAIR live in one
    128-partition tile (head A on partitions 0-63, head B on 64-127), so
    the K=64 score matmuls auto-pack as 64x128 row tiles of the PE array.
  * v = x @ w_v -> [2048, 768] natural layout (keys on partitions), which
    is exactly the lhsT needed for the attnV matmuls.
  * scoresT[m, n] = kT^T qT per head: keys on partitions, queries on the
    free dim.  exp() runs on ScalarE straight out of PSUM at FD=1024 (a
    head pair's [128, 2x512] chunk per instruction), with the 1/8
    softmax scale folded into the activation's free affine.  No max
    subtraction: scaled scores are ~N(0,1) so exp never overflows.
  * attnV: outT_h = v_h^T @ A_T^h accumulated over the 16 key tiles.  The
    two heads of a pair auto-pack as 128x64 column tiles (head A ->
    PSUM partitions 0-63, head B -> 64-127) sharing one PSUM bank.
  * softmax denominators: ones^T @ A_T matmuls, four heads (a "quad")
    packed as 128x32 column tiles into one PSUM bank.
  * normalization (divide by denominators) is applied at the attnV
    PSUM->SBUF eviction: reciprocal on DVE, broadcast across partitions
    via a DMA bounce, one tensor_tensor multiply.
  * final = outT^T @ w_proj with b_proj preloaded into PSUM by a K=1
    ones-matmul, evicted f32 and DMA'd out.
"""

import os

import numpy as np

import concourse.bass as bass
import concourse.mybir as mybir
from concourse import bacc, bass_utils
from concourse.tile import TileContext

F32 = mybir.dt.float32
BF16 = mybir.dt.bfloat16
AF = mybir.ActivationFunctionType

B, N, C = 8, 2048, 768
H, HD = 12, 64
SCALE = HD ** -0.5  # folded into the exp activation
P = 128
NT = N // P          # 16 token tiles
CT = C // P          # 6 feature tiles
NCHUNK = 4           # query chunks of 512
QW = N // NCHUNK     # 512


def build_nc() -> bass.Bass:
    nc = bacc.Bacc(None)
    x = nc.declare_dram_parameter("x", [N, C], F32, isOutput=False)
    w_qkv = nc.declare_dram_parameter("w_qkv", [C, 3 * C], F32, isOutput=False)
    w_proj = nc.declare_dram_parameter("w_proj", [C, C], F32, isOutput=False)
    b_proj = nc.declare_dram_parameter("b_proj", [C], F32, isOutput=False)
    out = nc.declare_dram_parameter("out", [N, C], F32, isOutput=True)

    with TileContext(nc) as tc:
        with (
            tc.tile_pool(name="const", bufs=1) as cpool,
            tc.tile_pool(name="dram", bufs=1, space="DRAM") as dpool,
            tc.tile_pool(name="rdram", bufs=2, space="DRAM") as rdpool,
            tc.tile_pool(name="at", bufs=6) as at_pool,
            tc.tile_pool(name="recip", bufs=2) as recip_pool,
            tc.tile_pool(name="rbc", bufs=2) as rbc_pool,
            tc.tile_pool(name="fin", bufs=2) as fin_pool,
            tc.tile_pool(name="psc", bufs=2, space="PSUM") as psum_sc,
            tc.tile_pool(name="pav", bufs=2, space="PSUM") as psum_av,
            tc.tile_pool(name="psum1", bufs=1, space="PSUM") as psum_sums,
            tc.tile_pool(name="pproj", bufs=1, space="PSUM") as psum_proj,
        ):
            # ---- persistent SBUF tensors -------------------------------
            w_qkv_sb = cpool.tile([P, CT, 3 * C], BF16, tag="wqkv")
            wproj_sb = cpool.tile([P, CT, C], BF16, tag="wproj")
            b_bc = cpool.tile([P, C], F32, tag="bias")  # bias bcast to 128 rows
            ones128 = cpool.tile([P, 1], BF16, tag="ones128")
            xT = cpool.tile([P, CT, N], BF16, tag="xT")
            qkT = cpool.tile([P, 12, N], BF16, tag="qkT")  # 12 = q(6 pairs)+k(6)
            v4 = cpool.tile([P, NT, C], BF16, tag="v4")
            outT = cpool.tile([P, CT, N], BF16, tag="outT")

            # ---- phase 0: load + cast + transpose ----------------------
            # interleave the x-cast chain with per-chunk w_qkv casts on the
            # SWDGE queue so the first qkT matmul's inputs (xT ct0 + w ct0)
            # are both ready within a few us; w_proj/bias load last.
            nc.any.memset(ones128[:], 1.0)
            nc.gpsimd.dma_start(
                out=w_qkv_sb[:], in_=w_qkv.rearrange("(o p) j -> p o j", p=P)
            )
            nc.gpsimd.dma_start(
                out=wproj_sb[:], in_=w_proj.rearrange("(o p) j -> p o j", p=P)
            )
            nc.sync.dma_start(
                out=b_bc[:], in_=b_proj[None, :].to_broadcast((P, C))
            )
            x_bf = dpool.tile([N, C], BF16)
            for ct in range(CT):
                csl = slice(ct * P, (ct + 1) * P)
                # per-column-chunk cast so each transpose starts early
                nc.gpsimd.dma_start(out=x_bf[:, csl], in_=x[:, csl])
                nc.sync.dma_start_transpose(xT[:, ct, :], x_bf[:, csl])

            # ---- phase 1: qkv projections ------------------------------
            # qkT[j, n] for j in [0, 1536): q rows 0-767, k rows 768-1535
            def emit_qk_group(jt: int, c4: int):
                ps = psum_sc.tile([P, 1024], F32, tag="sc")
                for ct in range(CT):
                    nc.tensor.matmul(
                        ps[:, 0:QW],
                        lhsT=w_qkv_sb[:, ct, jt * P : (jt + 1) * P],
                        rhs=xT[:, ct, c4 * QW : (c4 + 1) * QW],
                        start=(ct == 0),
                        stop=(ct == CT - 1),
                    )
                nc.vector.tensor_copy(
                    out=qkT[:, jt, c4 * QW : (c4 + 1) * QW], in_=ps[:, 0:QW]
                )

            # v natural layout: v[n, e] = sum_c x[n, c] w_qkv[c, 1536 + e]
            def emit_v_group(nt: int, eo: int, ew: int):
                ps = psum_sc.tile([P, 1024], F32, tag="sc")
                for ct in range(CT):
                    nc.tensor.matmul(
                        ps[:, 0:ew],
                        lhsT=xT[:, ct, nt * P : (nt + 1) * P],
                        rhs=w_qkv_sb[:, ct, 2 * C + eo : 2 * C + eo + ew],
                        start=(ct == 0),
                        stop=(ct == CT - 1),
                    )
                nc.vector.tensor_copy(out=v4[:, nt, eo : eo + ew], in_=ps[:, 0:ew])

            # upfront: only quad 0's needs — kT for pairs 0/1 and their
            # chunk-0 qT.  Everything else (later quads' kT/qT, v tiles,
            # later chunks' qT) is emitted just-in-time inside the attention
            # loops so ScalarE starts exping within ~15us of kernel start.
            for c4 in range(NCHUNK):
                emit_qk_group(6, c4)
            for c4 in range(NCHUNK):
                emit_qk_group(7, c4)
            emit_qk_group(0, 0)
            emit_qk_group(1, 0)
            # chunk-0 quad q prefetches quad q+1's kT (8 groups) + qT (2)
            c0_slots = {
                1: ("k", 0, 0), 2: ("k", 0, 1), 3: ("k", 0, 2), 4: ("k", 0, 3),
                5: ("k", 1, 0), 6: ("k", 1, 1), 7: ("k", 1, 2), 8: ("k", 1, 3),
                9: ("q", 0, 0), 10: ("q", 1, 0),
            }
            # chunk c's qT groups are emitted during chunk c-1, quad 2
            qt_slots = {2: 0, 5: 1, 8: 2, 11: 3, 13: 4, 15: 5}  # m -> jt

            # ---- phase 2+3: attention + projection ---------------------
            def emit_proj_group(nt: int, eo: int, ew: int):
                """final[nt-tile, eo:eo+ew] = outT^T w_proj + b."""
                ps = psum_proj.tile([P, 512], F32, tag="proj")
                for ct in range(CT):
                    nc.tensor.matmul(
                        ps[:, 0:ew],
                        lhsT=outT[:, ct, nt * P : (nt + 1) * P],
                        rhs=wproj_sb[:, ct, eo : eo + ew],
                        start=(ct == 0),
                        stop=(ct == CT - 1),
                    )
                fs = fin_pool.tile([P, 512], F32, tag="fin")
                nc.vector.tensor_tensor(
                    fs[:, 0:ew], ps[:, 0:ew], b_bc[:, eo : eo + ew],
                    mybir.AluOpType.add,
                )
                nc.sync.dma_start(
                    out=out[nt * P : (nt + 1) * P, eo : eo + ew], in_=fs[:, 0:ew]
                )

            # proj work for chunk c-1 is spread through chunk c's m-loops
            # (slots on quad 0/1 at fixed m) to avoid starving ScalarE.
            proj_slots = {  # (quad, m) -> slot index 0..7
                (0, 3): 0, (0, 7): 1, (0, 11): 2, (0, 14): 3,
                (1, 3): 4, (1, 7): 5, (1, 11): 6, (1, 14): 7,
            }

            def emit_proj_slot(c_done: int, slot: int):
                nt = c_done * 4 + slot // 2
                eo, ew = ((0, 512), (512, 256))[slot % 2]
                emit_proj_group(nt, eo, ew)

            for c in range(NCHUNK):
                qsl = slice(c * QW, (c + 1) * QW)
                for quad in range(3):
                    attn_ps = [
                        psum_av.tile([P, QW], F32, tag="av", name=f"av{pp}")
                        for pp in range(2)
                    ]
                    sums_ps = psum_sums.tile([P, QW], F32, tag="sums")
                    # only rows {0,32,64,96} get matmul results; init the rest
                    # so the full-tile reciprocal below reads defined memory
                    nc.vector.memset(sums_ps[:], 1.0)
                    for m in range(NT):
                        msl = slice(m * P, (m + 1) * P)
                        # just-in-time work: chunk 0 emits exactly the v
                        # columns this quad's attnV consumes, plus the next
                        # quad's kT/qT; quad 2 prefetches next chunk's qT.
                        if c == 0:
                            emit_v_group(m, quad * 256, 256)
                            if quad < 2 and m in c0_slots:
                                kind, i, c4s = c0_slots[m]
                                if kind == "k":
                                    emit_qk_group(8 + 2 * quad + i, c4s)
                                else:
                                    emit_qk_group(2 + 2 * quad + i, 0)
                        if quad == 2 and c < NCHUNK - 1 and m in qt_slots:
                            emit_qk_group(qt_slots[m], c + 1)
                        at_pair = []
                        for pp in range(2):
                            pair = 2 * quad + pp
                            sc = psum_sc.tile([P, 1024], F32, tag="sc")
                            # scoresT chunk: keys msl on partitions, queries
                            # qsl on free dim.  Head A rows 0-63, head B
                            # rows 64-127 -> auto row-tiled 64x128 pair.
                            nc.tensor.matmul(
                                sc[:, 0:QW],
                                lhsT=qkT[0:64, 6 + pair, msl],
                                rhs=qkT[0:64, pair, qsl],
                                start=True,
                                stop=True,
                            )
                            nc.tensor.matmul(
                                sc[:, QW : 2 * QW],
                                lhsT=qkT[64:128, 6 + pair, msl],
                                rhs=qkT[64:128, pair, qsl],
                                start=True,
                                stop=True,
                            )
                            at = at_pool.tile([P, 1024], BF16, tag="at")
                            nc.scalar.activation(at[:], sc[:], AF.Exp, scale=SCALE)
                            at_pair.append(at)
                        for pp in range(2):
                            pair = 2 * quad + pp
                            at = at_pair[pp]
                            for hh in range(2):
                                h = 2 * pair + hh
                                # attnV: col-tiled head pair, one PSUM bank
                                nc.tensor.matmul(
                                    attn_ps[pp][hh * 64 : (hh + 1) * 64, :],
                                    lhsT=v4[:, m, h * 64 : (h + 1) * 64],
                                    rhs=at[:, hh * QW : (hh + 1) * QW],
                                    start=(m == 0),
                                    stop=(m == NT - 1),
                                    # the sim's group-check view is partition-
                                    # blind; only the first col tile of the
                                    # shared bank may do the bookkeeping
                                    skip_group_check=(hh != 0),
                                )
                        for pp in range(2):
                            at = at_pair[pp]
                            for hh in range(2):
                                k4 = 2 * pp + hh
                                # denominators: 4 heads as 128x32 col tiles
                                nc.tensor.matmul(
                                    sums_ps[k4 * 32 : k4 * 32 + 1, :],
                                    lhsT=ones128[:, 0:1],
                                    rhs=at[:, hh * QW : (hh + 1) * QW],
                                    start=(m == 0),
                                    stop=(m == NT - 1),
                                    skip_group_check=(k4 != 0),
                                    tile_position=(0, k4 * 32),
                                )
                        if c > 0 and (quad, m) in proj_slots:
                            emit_proj_slot(c - 1, proj_slots[(quad, m)])

                    # ---- normalize + evict this (quad, chunk) ----------
                    recip_sb = recip_pool.tile([P, QW], F32, tag="recip")
                    nc.vector.reciprocal(recip_sb[:], sums_ps[:])
                    # bounce the 4 live rows through DRAM so a DMA can
                    # broadcast them across partitions
                    r_dram = rdpool.tile([4, QW], F32)
                    nc.sync.dma_start(out=r_dram[:], in_=recip_sb[0:97:32, :])
                    for pp in range(2):
                        rbc = rbc_pool.tile([P, QW], F32, tag="rbc")
                        nc.sync.dma_start(
                            out=rbc[0:64, :],
                            in_=r_dram[2 * pp : 2 * pp + 1, :].to_broadcast((64, QW)),
                        )
                        nc.sync.dma_start(
                            out=rbc[64:128, :],
                            in_=r_dram[2 * pp + 1 : 2 * pp + 2, :].to_broadcast(
                                (64, QW)
                            ),
                        )
                        nc.vector.tensor_tensor(
                            outT[:, 2 * quad + pp, qsl],
                            attn_ps[pp][:],
                            rbc[:],
                            mybir.AluOpType.mult,
                        )
            # tail: proj for the last chunk
            for slot in range(8):
                emit_proj_slot(NCHUNK - 1, slot)

    nc.compile()
    return nc


_NC_CACHE: list = []


def _get_nc() -> bass.Bass:
    if not _NC_CACHE:
        _NC_CACHE.append(build_nc())
    return _NC_CACHE[0]


def run(inputs: dict, trace: bool = False):
    """Run on 8 NeuronCores.  Returns (out [B,N,C] f32, exec_time_ns|None)."""
    nc = _get_nc()
    x = np.ascontiguousarray(np.asarray(inputs["x"], dtype=np.float32))
    w_qkv = np.ascontiguousarray(np.asarray(inputs["w_qkv"], dtype=np.float32))
    w_proj = np.ascontiguousarray(np.asarray(inputs["w_proj"], dtype=np.float32))
    b_proj = np.ascontiguousarray(np.asarray(inputs["b_proj"], dtype=np.float32))
    in_maps = [
        {"x": x[i], "w_qkv": w_qkv, "w_proj": w_proj, "b_proj": b_proj}
        for i in range(B)
    ]
    try:
        res = bass_utils.run_bass_kernel_spmd(
            nc, in_maps, core_ids=list(range(B)), trace=trace
        )
    except ModuleNotFoundError:
        # NTFF profile hook unavailable in this image; run without trace
        res = bass_utils.run_bass_kernel_spmd(
            nc, in_maps, core_ids=list(range(B)), trace=False
        )
    out = np.stack([res.results[i]["out"] for i in range(B)], axis=0)
    return out.astype(np.float32), res.exec_time_ns


def kernel(x, w_qkv, w_proj, b_proj):
    trace = os.environ.get("BASS_KERNEL_TRACE", "0") == "1"
    out, _ = run(
        {"x": x, "w_qkv": w_qkv, "w_proj": w_proj, "b_proj": b_proj}, trace=trace
    )
    return out



# revision 6
# speedup vs baseline: 1.5409x; 1.5409x over previous
"""Trainium2 Bass kernel for multi-head attention (dense transformer block).

Reference computation (per batch element):
    qkv = x @ w_qkv                      # [N, 3C]
    q, k, v = split heads (H=12, HD=64)
    out = softmax(q k^T * HD**-0.5) v    # full [N, N] scores
    out = merge_heads(out) @ w_proj + b_proj

Distribution: pure data parallel over the batch dim — B=8 batch elements,
8 NeuronCores, one element per core.  Weights are replicated.  No
collectives; each core computes its full [2048, 768] output.

Per-core design (cost-model-driven; matmul cost = out-free-size rows):
  * xT [768, 2048] bf16 via cast DMA + DMA-xbar transpose (as before).
  * qkT[j, n]: q/k for a head pair packed on 128 partitions (head A rows
    0-63, head B rows 64-127).
  * scoresT per (chunk c of 512 queries, pair, key-tile m): keys on
    partitions, queries free.  [128, 1024] (2 heads x 512 q) per m.
  * exp on TWO engines: ScalarE (ACT) and GpSimd (Pool) both run
    InstActivation(Exp, scale=1/8); tiles alternate 5:3 so neither is a
    bottleneck.  Output at [128, 1024] bf16.
  * attnV uses `at` as the STATIONARY side: lhsT = at[:, 128-query
    slice] (M=128), rhs = [v_h | ones] [128, 65] -> out [128 q, 65]
    where col 64 accumulates the softmax denominator.  8 matmuls of
    N=65 per m-step (8x65=520 rows vs 2048 in the v-stationary form,
    and the ones column makes the separate denominator matmuls free).
  * per (c, pair): two 1-bank PSUM accumulators (head A/B), 4 query
    groups x 65 cols each; after the 16-m sweep: DVE reciprocal of the
    D columns, then 8 tensor_scalar multiplies (per-partition scalar =
    recip) evict normalized [q, feat] bf16 tiles.
  * outQ [q, feat] bounced to DRAM and DMA-xbar transposed into
    outT [768, 2048] (feature-on-partition) for the projection.
  * projection + bias (DVE add) unchanged; proj for chunk c-1 is slotted
    through chunk c's m-stream; the whole attention is one flat
    software-pipelined stream (attnV lags scores by one m-step).
"""

import os

import numpy as np

import concourse.bass as bass
import concourse.mybir as mybir
from concourse import bacc, bass_utils
from concourse.tile import TileContext

F32 = mybir.dt.float32
BF16 = mybir.dt.bfloat16
AF = mybir.ActivationFunctionType
IMM = mybir.ImmediateValue

B, N, C = 8, 2048, 768
H, HD = 12, 64
SCALE = HD ** -0.5  # folded into the exp activation
P = 128
NT = N // P          # 16 key tiles
CT = C // P          # 6 feature tiles
NCHUNK = 4           # query chunks of 512
QW = N // NCHUNK     # 512
PAIRS = 6            # head pairs
VW = HD + 1          # 65: v columns + ones column (denominator)


def _activation_on(nc, eng, out, in_, func, bias=0.0, scale=1.0):
    """InstActivation emitted on an arbitrary engine (ACT or Pool)."""
    if isinstance(bias, float) and func not in (AF.Copy, AF.Reciprocal):
        bias = nc.const_aps.scalar_like(bias, in_)
    ins = [eng.lower_ap(in_)]
    for arg in (bias, scale, 0.0):
        if isinstance(arg, bass.AP):
            ins.append(eng.lower_ap(arg))
        else:
            ins.append(IMM(dtype=mybir.dt.float32, value=arg))
    return eng.add_instruction(
        mybir.InstActivation(
            name=nc.get_next_instruction_name(),
            func=func,
            ins=ins,
            outs=[eng.lower_ap(out)],
        )
    )


def build_nc() -> bass.Bass:
    nc = bacc.Bacc(None)
    x = nc.declare_dram_parameter("x", [N, C], F32, isOutput=False)
    w_qkv = nc.declare_dram_parameter("w_qkv", [C, 3 * C], F32, isOutput=False)
    w_proj = nc.declare_dram_parameter("w_proj", [C, C], F32, isOutput=False)
    b_proj = nc.declare_dram_parameter("b_proj", [C], F32, isOutput=False)
    out = nc.declare_dram_parameter("out", [N, C], F32, isOutput=True)

    with TileContext(nc) as tc:
        with (
            tc.tile_pool(name="const", bufs=1) as cpool,
            tc.tile_pool(name="dram", bufs=1, space="DRAM") as dpool,
            tc.tile_pool(name="oqdram", bufs=2, space="DRAM") as oqd_pool,
            tc.tile_pool(name="at", bufs=6) as at_pool,
            tc.tile_pool(name="oq", bufs=2) as oq_pool,
            tc.tile_pool(name="recip", bufs=2) as recip_pool,
            tc.tile_pool(name="fin", bufs=2) as fin_pool,
            tc.tile_pool(name="psc", bufs=2, space="PSUM") as psum_sc,
            tc.tile_pool(name="pav", bufs=3, space="PSUM") as psum_av,
            tc.tile_pool(name="pproj", bufs=1, space="PSUM") as psum_proj,
        ):
            # ---- persistent SBUF tensors -------------------------------
            w_qkv_sb = cpool.tile([P, CT, 3 * C], BF16, tag="wqkv")
            wproj_sb = cpool.tile([P, CT, C], BF16, tag="wproj")
            b_bc = cpool.tile([P, C], F32, tag="bias")  # bias bcast to 128 rows
            xT = cpool.tile([P, CT, N], BF16, tag="xT")
            qkT = cpool.tile([P, 12, N], BF16, tag="qkT")  # q pairs 0-5, k 6-11
            vp = cpool.tile([P, NT, H * VW], BF16, tag="vp")  # [v_h | 1] per head
            outT = cpool.tile([P, PAIRS, N], BF16, tag="outT")

            # ---- phase 0: load + cast + transpose ----------------------
            nc.vector.memset(vp[:, :, HD :: VW], 1.0)  # ones cols (denominator)
            nc.gpsimd.dma_start(
                out=w_qkv_sb[:], in_=w_qkv.rearrange("(o p) j -> p o j", p=P)
            )
            nc.gpsimd.dma_start(
                out=wproj_sb[:], in_=w_proj.rearrange("(o p) j -> p o j", p=P)
            )
            nc.sync.dma_start(
                out=b_bc[:], in_=b_proj[None, :].to_broadcast((P, C))
            )
            x_bf = dpool.tile([N, C], BF16)
            for ct in range(CT):
                csl = slice(ct * P, (ct + 1) * P)
                nc.gpsimd.dma_start(out=x_bf[:, csl], in_=x[:, csl])
                nc.sync.dma_start_transpose(xT[:, ct, :], x_bf[:, csl])

            # ---- emit helpers ------------------------------------------
            def emit_qk_group(jt: int, c4: int):
                """qkT[:, jt, c4*512:(c4+1)*512] = (w_qkv col block)^T x^T."""
                ps = psum_sc.tile([P, 1024], F32, tag="sc")
                for ct in range(CT):
                    nc.tensor.matmul(
                        ps[:, 0:QW],
                        lhsT=w_qkv_sb[:, ct, jt * P : (jt + 1) * P],
                        rhs=xT[:, ct, c4 * QW : (c4 + 1) * QW],
                        start=(ct == 0),
                        stop=(ct == CT - 1),
                    )
                nc.vector.tensor_copy(
                    out=qkT[:, jt, c4 * QW : (c4 + 1) * QW], in_=ps[:, 0:QW]
                )

            def emit_v_group(nt: int, p: int):
                """vp[:, nt, pair-p head cols] = x-tile @ w_v (natural layout)."""
                ps = psum_proj.tile([P, 512], F32, tag="proj")
                for ct in range(CT):
                    nc.tensor.matmul(
                        ps[:, 0:P],
                        lhsT=xT[:, ct, nt * P : (nt + 1) * P],
                        rhs=w_qkv_sb[:, ct, 2 * C + p * P : 2 * C + (p + 1) * P],
                        start=(ct == 0),
                        stop=(ct == CT - 1),
                    )
                # scatter the two heads' 64-col halves into the 65-col slots
                nc.vector.tensor_copy(
                    out=vp[:, nt, 2 * p * VW : 2 * p * VW + 2 * VW].rearrange(
                        "p (h w) -> p h w", h=2
                    )[:, :, 0:HD],
                    in_=ps[:, 0:P].rearrange("p (h w) -> p h w", h=2),
                )

            def emit_proj_group(nt: int, eo: int, ew: int):
                """final[nt-tile, eo:eo+ew] = outT^T w_proj + b."""
                ps = psum_proj.tile([P, 512], F32, tag="proj")
                for ct in range(CT):
                    nc.tensor.matmul(
                        ps[:, 0:ew],
                        lhsT=outT[:, ct, nt * P : (nt + 1) * P],
                        rhs=wproj_sb[:, ct, eo : eo + ew],
                        start=(ct == 0),
                        stop=(ct == CT - 1),
                    )
                fs = fin_pool.tile([P, 512], F32, tag="fin")
                nc.vector.tensor_tensor(
                    fs[:, 0:ew], ps[:, 0:ew], b_bc[:, eo : eo + ew],
                    mybir.AluOpType.add,
                )
                nc.sync.dma_start(
                    out=out[nt * P : (nt + 1) * P, eo : eo + ew], in_=fs[:, 0:ew]
                )

            def emit_proj_slot(c_done: int, slot: int):
                nt = c_done * 4 + slot // 2
                eo, ew = ((0, 512), (512, 256))[slot % 2]
                emit_proj_group(nt, eo, ew)

            # ---- phase 1 upfront: pair-0 kT + qT(0, chunk0) -------------
            for c4 in range(NCHUNK):
                emit_qk_group(6, c4)
            emit_qk_group(0, 0)

            # ---- phase 2: flat software-pipelined attention stream ------
            # per (c, p): m-sweep over 16 key tiles; attnV lags scores by
            # one step so the PE never waits on the exp engines.
            kt_slots = {1: 0, 4: 1, 7: 2, 10: 3}  # m -> c4 of kT(p+1)
            proj_slots = {  # (p, m) -> slot
                (1, 3): 0, (1, 11): 1, (2, 3): 2, (2, 11): 3,
                (3, 3): 4, (3, 11): 5, (4, 3): 6, (4, 11): 7,
            }

            state = {"i": 0}  # exp tile counter for engine assignment

            def emit_scores(c, p, m):
                qsl = slice(c * QW, (c + 1) * QW)
                msl = slice(m * P, (m + 1) * P)
                sc = psum_sc.tile([P, 1024], F32, tag="sc")
                nc.tensor.matmul(
                    sc[:, 0:QW],
                    lhsT=qkT[0:64, 6 + p, msl],
                    rhs=qkT[0:64, p, qsl],
                    start=True,
                    stop=True,
                )
                nc.tensor.matmul(
                    sc[:, QW : 2 * QW],
                    lhsT=qkT[64:128, 6 + p, msl],
                    rhs=qkT[64:128, p, qsl],
                    start=True,
                    stop=True,
                )
                at = at_pool.tile([P, 1024], BF16, tag="at")
                i = state["i"]
                state["i"] = i + 1
                eng = nc.scalar if os.environ.get("NO_POOL_EXP") else (nc.scalar if i % 8 in (0, 2, 3, 5, 6) else nc.gpsimd)
                _activation_on(nc, eng, at[:], sc[:], AF.Exp, scale=SCALE)
                return at

            def emit_attnv(av_banks, at, p, m):
                for hh in range(2):
                    h = 2 * p + hh
                    for g in range(4):
                        # start zeroes the whole 2KB zero region (the bank),
                        # so only the first col-group starts and only the
                        # last one stops the accumulation group
                        nc.tensor.matmul(
                            av_banks[hh][:, g * VW : (g + 1) * VW],
                            lhsT=at[:, hh * QW + g * P : hh * QW + (g + 1) * P],
                            rhs=vp[:, m, h * VW : (h + 1) * VW],
                            start=(m == 0 and g == 0),
                            stop=(m == NT - 1 and g == 3),
                        )

            def emit_evict(av_banks, c, p):
                """normalize by the accumulated denominators and store outQ,
                then bounce to DRAM and DMA-transpose into outT."""
                rc = recip_pool.tile([P, 8], F32, tag="recip")
                for hh in range(2):
                    nc.vector.reciprocal(
                        rc[:, 4 * hh : 4 * hh + 4],
                        av_banks[hh][:, HD : 4 * VW : VW],
                    )
                oq = oq_pool.tile([P, 4, P], BF16, tag="oq")
                for hh in range(2):
                    for g in range(4):
                        nc.vector.tensor_scalar(
                            out=oq[:, g, hh * HD : (hh + 1) * HD],
                            in0=av_banks[hh][:, g * VW : g * VW + HD],
                            scalar1=rc[:, 4 * hh + g : 4 * hh + g + 1],
                            scalar2=None,
                            op0=mybir.AluOpType.mult,
                        )
                oqd = oqd_pool.tile([QW, P], BF16)
                nc.sync.dma_start(
                    out=oqd.rearrange("(a p) f -> p a f", p=P), in_=oq[:]
                )
                nc.sync.dma_start_transpose(
                    outT[:, p, c * QW : (c + 1) * QW], oqd[:]
                )

            steps = [
                (c, p, m)
                for c in range(NCHUNK)
                for p in range(PAIRS)
                for m in range(NT)
            ]
            prev = None  # (av_banks, at, c, p, m)
            av_banks = None
            for (c, p, m) in steps:
                if m == 0:
                    av_banks = [
                        psum_av.tile([P, 512], F32, tag="av", name=f"av{c}_{p}_{hh}")
                        for hh in range(2)
                    ]
                # ---- JIT slots ----
                if c == 0:
                    emit_v_group(m, p)
                    if p < PAIRS - 1:
                        if m in kt_slots:
                            emit_qk_group(6 + p + 1, kt_slots[m])
                        elif m == 13:
                            emit_qk_group(p + 1, 0)
                if m == 8 and c < NCHUNK - 1:
                    emit_qk_group(p, c + 1)  # next chunk's qT for this pair
                if c > 0 and (p, m) in proj_slots:
                    emit_proj_slot(c - 1, proj_slots[(p, m)])
                # ---- scores + exp for this step ----
                at = emit_scores(c, p, m)
                cur = (av_banks, at, c, p, m)
                # ---- attnV for the previous step (one-step lag) ----
                if prev is not None:
                    pav, pat, pc, pp, pm = prev
                    emit_attnv(pav, pat, pp, pm)
                    if pm == NT - 1:
                        emit_evict(pav, pc, pp)
                prev = cur
            pav, pat, pc, pp, pm = prev
            emit_attnv(pav, pat, pp, pm)
            emit_evict(pav, pc, pp)
            # tail: proj for the last chunk
            for slot in range(8):
                emit_proj_slot(NCHUNK - 1, slot)

    nc.compile()
    return nc


_NC_CACHE: list = []


def _get_nc() -> bass.Bass:
    if not _NC_CACHE:
        _NC_CACHE.append(build_nc())
    return _NC_CACHE[0]


def run(inputs: dict, trace: bool = False):
    """Run on 8 NeuronCores.  Returns (out [B,N,C] f32, exec_time_ns|None)."""
    nc = _get_nc()
    x = np.ascontiguousarray(np.asarray(inputs["x"], dtype=np.float32))
    w_qkv = np.ascontiguousarray(np.asarray(inputs["w_qkv"], dtype=np.float32))
    w_proj = np.ascontiguousarray(np.asarray(inputs["w_proj"], dtype=np.float32))
    b_proj = np.ascontiguousarray(np.asarray(inputs["b_proj"], dtype=np.float32))
    in_maps = [
        {"x": x[i], "w_qkv": w_qkv, "w_proj": w_proj, "b_proj": b_proj}
        for i in range(B)
    ]
    try:
        res = bass_utils.run_bass_kernel_spmd(
            nc, in_maps, core_ids=list(range(B)), trace=trace
        )
    except ModuleNotFoundError:
        res = bass_utils.run_bass_kernel_spmd(
            nc, in_maps, core_ids=list(range(B)), trace=False
        )
    out = np.stack([res.results[i]["out"] for i in range(B)], axis=0)
    return out.astype(np.float32), res.exec_time_ns


def kernel(x, w_qkv, w_proj, b_proj):
    trace = os.environ.get("BASS_KERNEL_TRACE", "0") == "1"
    out, _ = run(
        {"x": x, "w_qkv": w_qkv, "w_proj": w_proj, "b_proj": b_proj}, trace=trace
    )
    return out
